# revision 2
# baseline (speedup 1.0000x reference)
"""LinearAttention kernel for Trainium2, 8 NeuronCores, data-parallel over batch.

v2: fp8 DoubleRow matmuls; ACT runs only {Exp, Square, Copy} (single act table);
rsqrt/divide done on DVE via pow/divide ALU ops; per-token reductions computed
compactly ([128,4] per tile) and replicated through tiny PE outer-products.

Scaling bookkeeping (all cancel in the final rms-norm):
  Wqkv stored *16 (fp8 range), exp() applied with scale=1/16, bias=-1.5.
  q_sm stored *16 (block-diag mask folds 1/16 into the replicated qden).
  W2 stored = (Wout @ ctx^T / kden) * 0.5  (= 64 * W2_true; v carries its 16)
  => y psum = 1024 * y_true; bout row scaled *1024; rms-norm removes it all.

Per-batch layouts:
  x, xn     [c=2x128, n]    channels on partitions (xn fp8)
  q psum    [128, TN]       per ob; expq fp32r SBUF; q_sm fp8 = expq/qdrep
  kT/vT     [tok 128, 512]  per 128-token block (k and v each one DR matmul)
  ctx psum  [66, 8*64]      rows 0..63 = sum exp(k)*v, row 64 = kden
  y psum    [c 2x128, TN]   stage B in [c, n]; per-token rsqrt replicated via
                            transpose + outer-products; final scale on Pool.
"""

import numpy as np

import concourse.bass as bass
import concourse.tile as tile
from concourse import bacc, mybir
from concourse.bass_utils import run_bass_kernel_spmd
from concourse.masks import make_identity

F32 = mybir.dt.float32
F32R = mybir.dt.float32r
BF16 = mybir.dt.bfloat16
FP8 = mybir.dt.float8e4
U32 = mybir.dt.uint32

AF = mybir.ActivationFunctionType
ALU = mybir.AluOpType
DR = mybir.MatmulPerfMode.DoubleRow

B = 16          # total batches
BL = 2          # batches per core
C = 256         # in channels
HID = 512       # heads * dim_head
HEADS = 8
DH = 64         # dim head
N = 4096        # tokens (64*64)
TN = 512        # token tile
NT = N // TN    # 8 token tiles per batch
NB = TN // 128  # 4 128-token blocks per tile


def build_kernel():
    nc = bacc.Bacc("TRN2", target_bir_lowering=False, debug=False, num_devices=8)

    x_d = nc.dram_tensor("x", [BL, C, N], F32, kind="ExternalInput").ap()
    wqkv_d = nc.dram_tensor("Wqkv", [3 * HID, C], F32, kind="ExternalInput").ap()
    wout_d = nc.dram_tensor("Wout", [C, HID], F32, kind="ExternalInput").ap()
    bout_d = nc.dram_tensor("bout", [C], F32, kind="ExternalInput").ap()
    g1_d = nc.dram_tensor("g1", [C], F32, kind="ExternalInput").ap()
    g2_d = nc.dram_tensor("g2", [C], F32, kind="ExternalInput").ap()
    o_d = nc.dram_tensor("out", [BL, C, N], F32, kind="ExternalOutput").ap()

    xv = x_d.rearrange("b (cb p) n -> b p cb n", cb=2)
    ov = o_d.rearrange("b (cb p) n -> b p cb n", cb=2)

    with tile.TileContext(nc) as tc:
        with (
            tc.tile_pool(name="const", bufs=1) as const,
            tc.tile_pool(name="wt", bufs=1) as wt,
            tc.tile_pool(name="stage", bufs=1) as stage,
            tc.tile_pool(name="xin", bufs=3) as xin,
            tc.tile_pool(name="front", bufs=2) as front,
            tc.tile_pool(name="qwork", bufs=2) as qwork,
            tc.tile_pool(name="kvw", bufs=3) as kvw,
            tc.tile_pool(name="persist", bufs=2) as persist,
            tc.tile_pool(name="bwork", bufs=2) as bwork,
            tc.tile_pool(name="ps_a", bufs=4, space="PSUM") as ps_a,
            tc.tile_pool(name="ps_sm", bufs=1, space="PSUM") as ps_sm,
            tc.tile_pool(name="ps_kv", bufs=2, space="PSUM") as ps_kv,
            tc.tile_pool(name="ps_ctx", bufs=1, space="PSUM") as ps_ctx,
        ):
            # ---------------- constants ----------------
            ident = const.tile([128, 128], F32)
            make_identity(nc, ident)

            ones2_f8 = const.tile([128, 2, 1], FP8)
            nc.gpsimd.memset(ones2_f8, 1.0)
            onerow_bf = const.tile([1, 128], BF16)
            nc.gpsimd.memset(onerow_bf, 1.0)

            # block-diag [128,128] fp32r, value 1/16: replicated per-head sums
            bd_f = const.tile([128, 128], F32)
            nc.gpsimd.memset(bd_f, 0.0)
            nc.gpsimd.memset(bd_f[0:64, 0:64], 1.0 / 16.0)
            nc.gpsimd.memset(bd_f[64:128, 64:128], 1.0 / 16.0)
            bdr = const.tile([128, 128], BF16)
            nc.vector.tensor_copy(out=bdr, in_=bd_f)

            onecol_bf = const.tile([128, 1], BF16)
            nc.gpsimd.memset(onecol_bf, 1.0)
            onesrow = const.tile([1, 512], BF16)
            nc.gpsimd.memset(onesrow, 1.0)
            biasm = const.tile([128, 1], F32)
            nc.gpsimd.memset(biasm, -1.5)

            g1c = const.tile([128, 2], F32)
            nc.sync.dma_start(out=g1c, in_=g1_d.rearrange("(cb p) -> p cb", cb=2))
            g1s = const.tile([128, 2], F32)
            nc.vector.tensor_scalar_mul(out=g1s, in0=g1c, scalar1=16.0)
            g2c = const.tile([128, 2], F32)
            nc.sync.dma_start(out=g2c, in_=g2_d.rearrange("(cb p) -> p cb", cb=2))

            # bout as a row, scaled by 1024 (total y scale), bf16
            brow = const.tile([1, 256], F32)
            nc.sync.dma_start(out=brow, in_=bout_d.rearrange("(a c) -> a c", a=1))
            broww = const.tile([1, 256], BF16)
            nc.vector.tensor_scalar_mul(out=broww, in0=brow, scalar1=1024.0)

            # prefetch the first x tiles before the weight-prep flood
            xts_pre = {}
            for jpre in range(2):
                xt_pre = xin.tile([128, 2, TN], F32, tag="xt", name="xt_pre")
                nc.sync.dma_start(out=xt_pre, in_=xv[0, :, :, jpre * TN:(jpre + 1) * TN])
                xts_pre[jpre] = xt_pre

            # ---------------- weights ----------------
            # Wqkv [1536, 256] -> wqkvT fp8 [c(2x128), cb, 1536], rows *g1*16
            wq_nat = stage.tile([128, 12, 256], F32, tag="wnat")
            nc.sync.dma_start(
                out=wq_nat, in_=wqkv_d.rearrange("(ob p) c -> p ob c", p=128)
            )
            wqkvTb = wt.tile([128, 2, 1536], BF16)
            for ob in range(12):
                for cb in range(2):
                    pt = ps_a.tile([128, 512], F32, tag="pa", name="pt")
                    nc.tensor.transpose(
                        pt[:, 0:128], wq_nat[:, ob, cb * 128:(cb + 1) * 128], ident
                    )
                    nc.vector.tensor_scalar_mul(
                        out=wqkvTb[:, cb, ob * 128:(ob + 1) * 128],
                        in0=pt[:, 0:128],
                        scalar1=g1s[:, cb:cb + 1],
                    )
            # Wout [256, 512] -> woutT [e=64, h, 256] fp32r
            wo_nat = stage.tile([128, 2, 512], F32, tag="wnat")
            nc.sync.dma_start(
                out=wo_nat, in_=wout_d.rearrange("(ob p) h -> p ob h", p=128)
            )
            woutT = wt.tile([64, 8, 256], F32R)
            for h in range(HEADS):
                for ob in range(2):
                    pt = ps_a.tile([128, 512], F32, tag="pa", name="pt")
                    nc.tensor.transpose(
                        pt[0:64, 0:128], wo_nat[:, ob, h * 64:(h + 1) * 64], ident
                    )
                    nc.vector.tensor_copy(
                        out=woutT[:, h, ob * 128:(ob + 1) * 128], in_=pt[0:64, 0:128]
                    )
            scl_f = const.tile([1, 2], F32)
            nc.gpsimd.memset(scl_f, 0.5)
            scl_r = const.tile([1, 2], F32R)
            nc.vector.tensor_copy(out=scl_r, in_=scl_f)  # kden transpose helper

            # ---------------- helpers ----------------
            MAGIC = 0x5F3759DF + 0x02000000  # rsqrt seed for m = ssq/256
            MAGIC_SUB = 0x7FFFFFFF - MAGIC  # overflow-free: C-(i>>1) = ((i>>1)^0x7fffffff) - this

            def rsqrt_compact(cT, sm, pool, tag):
                """cT [128,4] f32 psum of per-token ssq -> replicated
                [128, 512] f32 psum with 16/sqrt(ssq) (bit-hack + 1 Newton).
                sm is the host psum tile for the transpose scratch."""
                t1 = pool.tile([128, 4], U32, tag=tag + "t1", name="t1")
                nc.vector.tensor_scalar(
                    out=t1, in0=cT.bitcast(U32), scalar1=1, scalar2=0x7FFFFFFF,
                    op0=ALU.logical_shift_right, op1=ALU.bitwise_xor,
                )
                y0 = pool.tile([128, 4], U32, tag=tag + "y0", name="y0")
                nc.vector.tensor_scalar(
                    out=y0, in0=t1, scalar1=MAGIC_SUB, scalar2=None, op0=ALU.subtract
                )
                y0f = y0.bitcast(F32)
                t2 = pool.tile([128, 4], F32, tag=tag + "t2", name="t2")
                nc.vector.tensor_mul(t2, y0f, y0f)
                nc.vector.tensor_mul(t2, t2, cT)
                t3 = pool.tile([128, 4], F32, tag=tag + "t3", name="t3")
                nc.vector.tensor_scalar(
                    out=t3, in0=t2, scalar1=-1.0 / 512.0, scalar2=1.5,
                    op0=ALU.mult, op1=ALU.add,
                )
                y1 = pool.tile([128, 4], F32, tag=tag + "y1", name="y1")
                nc.vector.tensor_mul(y1, y0f, t3)
                nc.vector.tensor_mul(t2, y1, y1)
                nc.vector.tensor_mul(t2, t2, cT)
                nc.vector.tensor_scalar(
                    out=t3, in0=t2, scalar1=-1.0 / 512.0, scalar2=1.5,
                    op0=ALU.mult, op1=ALU.add,
                )
                vT = pool.tile([128, 4], F32, tag=tag + "vT", name="vT")
                nc.vector.tensor_mul(vT, y1, t3)
                # replicate: transpose -> sbuf bf16 -> row-flatten -> outer
                tt = sm[0:4, 128:256]
                nc.tensor.transpose(tt, vT, ident)
                v_sb = pool.tile([4, 128], BF16, tag=tag + "sb", name="v_sb")
                nc.vector.tensor_copy(out=v_sb, in_=tt)
                vrow = pool.tile([1, 512], BF16, tag=tag + "row", name="vrow")
                nc.sync.dma_start(out=vrow, in_=v_sb)
                nc.tensor.matmul(sm, onerow_bf, vrow, start=True, stop=True)
                return sm

            def emit_load(bl, j):
                """Prefetch x tile j."""
                xt = xin.tile([128, 2, TN], F32, tag="xt")
                nc.sync.dma_start(out=xt, in_=xv[bl, :, :, j * TN:(j + 1) * TN])
                return xt

            def emit_front1(bl, j, xt):
                """x^2 + compact per-token ssq for tile j."""
                x2 = front.tile([128, 2, TN], FP8, tag="x2")
                nc.gpsimd.tensor_mul(x2, xt, xt)
                sm = ps_sm.tile([128, 512], F32, tag="sm", name="sm")
                for nb in range(NB):
                    nc.tensor.matmul(
                        sm[:, nb:nb + 1],
                        x2[:, :, nb * 128:(nb + 1) * 128],
                        ones2_f8,
                        start=True, stop=True,
                        perf_mode=DR,
                        skip_group_check=True,
                    )
                return sm

            def emit_front2(bl, j, xt, sm, xn_full):
                """rsqrt + normalize tile j into xn_full (fp8)."""
                t0 = j * TN
                sinvrep = rsqrt_compact(sm[:, 0:4], sm, front, "sinv")
                for cb in range(2):
                    nc.vector.tensor_mul(
                        xn_full[:, cb, t0:t0 + TN], xt[:, cb, :], sinvrep[:, 0:TN]
                    )

            def emit_qproj(bl, j, xn_full):
                """S1: q projection + exp for tile j -> (expq, pq psum tiles)."""
                t0 = j * TN
                expq = qwork.tile([128, 4, TN], BF16, tag="expq")
                pqs = []
                for ob in range(4):
                    pq = ps_a.tile([128, 512], F32, tag="pa", name="pq")
                    pqs.append(pq)
                    for cb in range(2):
                        nc.tensor.matmul(
                            pq[:, 0:TN],
                            wqkvTb[:, cb, ob * 128:(ob + 1) * 128],
                            xn_full[:, cb, t0:t0 + TN],
                            start=(cb == 0), stop=(cb == 1),
                        )
                    nc.scalar.activation(
                        out=expq[:, ob, :], in_=pq[:, 0:TN],
                        func=AF.Exp, scale=1.0 / 16.0, bias=biasm,
                    )
                return expq, pqs

            def emit_qdiv(bl, j, expq, pqs, q_sm):
                """S2: replicated per-head denominators (into the freed pq
                banks), reciprocal to SBUF, multiply on Pool."""
                t0 = j * TN
                for ob in range(4):
                    nc.tensor.matmul(
                        pqs[ob][:, 0:TN], bdr, expq[:, ob, :],
                        start=True, stop=True,
                    )
                    qdinv = qwork.tile([128, TN], F32R, tag="qdinv")
                    with nc.allow_low_precision(reason="q softmax denom"):
                        nc.vector.reciprocal(out=qdinv, in_=pqs[ob][:, 0:TN])
                    nc.gpsimd.tensor_mul(
                        q_sm[:, ob, t0:t0 + TN], expq[:, ob, :], qdinv
                    )

            def emit_kvpath(bl, j, xn_full, ctx):
                """kv projection + exp(k) + ctx accumulation for tile j."""
                t0 = j * TN
                for half in range(2):  # two 2-block pairs
                    expk2 = kvw.tile([128, 2, 8, 64], BF16, tag="expk")
                    vt2 = kvw.tile([128, 2, 8, 66], BF16, tag="vt")
                    nc.gpsimd.memset(vt2[:, :, :, 64:65], 1.0)
                    nc.gpsimd.memset(vt2[:, :, :, 65:66], 0.0)
                    for bi in range(2):
                        nb = half * 2 + bi
                        pk = ps_kv.tile([128, 512], F32, tag="pkv", name="pk")
                        for cb in range(2):
                            nc.tensor.matmul(
                                pk,
                                xn_full[:, cb, t0 + nb * 128:t0 + (nb + 1) * 128],
                                wqkvTb[:, cb, 512:1024],
                                start=(cb == 0), stop=(cb == 1),
                            )
                        pv = ps_kv.tile([128, 512], F32, tag="pkv", name="pv")
                        for cb in range(2):
                            nc.tensor.matmul(
                                pv,
                                xn_full[:, cb, t0 + nb * 128:t0 + (nb + 1) * 128],
                                wqkvTb[:, cb, 1024:1536],
                                start=(cb == 0), stop=(cb == 1),
                            )
                        nc.scalar.activation(
                            out=expk2[:, bi], in_=pk,
                            func=AF.Exp, scale=1.0 / 16.0, bias=biasm,
                        )
                        if half == 0:
                            nc.scalar.copy(
                                out=vt2[:, bi, :, 0:64],
                                in_=pv.rearrange("p (h e) -> p h e", h=8),
                            )
                        else:
                            nc.vector.tensor_copy(
                                out=vt2[:, bi, :, 0:64],
                                in_=pv.rearrange("p (h e) -> p h e", h=8),
                            )
                    gpair = j * 2 + half
                    for bi in range(2):
                        for h in range(HEADS):
                            nc.tensor.matmul(
                                ctx[0:66, h * 64:(h + 1) * 64],
                                vt2[:, bi, h, :],
                                expk2[:, bi, h, :],
                                start=False,
                                stop=(gpair == 2 * NT - 1 and bi == 1),
                                skip_group_check=True,
                            )

            def emit_epilogue(bl, ctx):
                """Build w2T fp8 [128, 4, 256] = (Wout_h @ ctx_h^T / kden * .5)^T"""
                ctx_sb = persist.tile([64, 512], F32R, tag="ctxsb")
                nc.vector.tensor_copy(out=ctx_sb, in_=ctx[0:64, :])
                kdrow = persist.tile([1, 512], F32R, tag="kdrow")
                with nc.allow_low_precision(reason="k softmax denominators"):
                    nc.vector.reciprocal(out=kdrow, in_=ctx[64:65, :])
                # transpose kden row -> columns [64, 8]
                pkd = ps_a.tile([128, 512], F32, tag="pa", name="pkd")
                for h in range(HEADS):
                    nc.tensor.matmul(
                        pkd[0:64, 2 * h:2 * h + 2],
                        kdrow[0:1, h * 64:(h + 1) * 64],
                        scl_r,
                        start=True, stop=True,
                        skip_group_check=True,
                    )
                kdcol = persist.tile([64, 8, 1], F32, tag="kdcol")
                pkd_v = pkd[0:64, 0:16].rearrange("p (h t) -> p h t", t=2)
                nc.vector.tensor_copy(out=kdcol, in_=pkd_v[:, :, 0:1])
                w2stg = persist.tile([64, 8, 256], BF16, tag="w2stg")
                for h in range(HEADS):
                    pw2 = ps_a.tile([128, 512], F32, tag="pa", name="pw2")
                    nc.tensor.matmul(
                        pw2[0:64, 0:256],
                        ctx_sb[:, h * 64:(h + 1) * 64],
                        woutT[:, h, :],
                        start=True, stop=True,
                    )
                    nc.vector.tensor_scalar_mul(
                        out=w2stg[:, h, :],
                        in0=pw2[0:64, 0:256],
                        scalar1=kdcol[:, h, :],
                    )
                w2T = persist.tile([128, 4, 256], BF16, tag="w2T")
                for h in range(HEADS):
                    nc.sync.dma_start(
                        out=w2T[(h % 2) * 64:(h % 2) * 64 + 64, h // 2, :],
                        in_=w2stg[:, h, :],
                    )
                return w2T

            def emit_b1(bl, j, q_sm, w2T):
                """Stage B matmuls + y^2 for tile j."""
                t0 = j * TN
                yB = [None, None]
                y2 = bwork.tile([128, 2, TN], BF16, tag="y2")
                for cb in range(2):
                    yB[cb] = ps_a.tile([128, 512], F32, tag="pa", name="yB")
                    for cp in range(4):
                        nc.tensor.matmul(
                            yB[cb][:, 0:TN],
                            w2T[:, cp, cb * 128:(cb + 1) * 128],
                            q_sm[:, cp, t0:t0 + TN],
                            start=(cp == 0), stop=False,
                            skip_group_check=True,
                        )
                    nc.tensor.matmul(
                        yB[cb][:, 0:TN],
                        broww[0:1, cb * 128:(cb + 1) * 128],
                        onesrow,
                        start=False, stop=True,
                        skip_group_check=True,
                    )
                    nc.scalar.activation(
                        out=y2[:, cb, :], in_=yB[cb][:, 0:TN], func=AF.Square
                    )
                smb = ps_sm.tile([128, 512], F32, tag="sm", name="smb")
                for nb in range(NB):
                    for cb in range(2):
                        nc.tensor.matmul(
                            smb[:, nb:nb + 1],
                            y2[:, cb, nb * 128:(nb + 1) * 128],
                            onecol_bf,
                            start=(cb == 0), stop=(cb == 1),
                            skip_group_check=True,
                        )
                return yB, smb

            def emit_b2(bl, j, yB, smb):
                """Stage B rms-out + store for tile j."""
                t0 = j * TN
                rrep_ps = rsqrt_compact(smb[:, 0:4], smb, bwork, "rinv")
                rinvrep = bwork.tile([128, TN], F32, tag="rinvsb")
                nc.scalar.copy(out=rinvrep, in_=rrep_ps[:, 0:TN])
                yout = bwork.tile([128, 2, TN], F32, tag="yout")
                for cb in range(2):
                    nc.vector.scalar_tensor_tensor(
                        out=yout[:, cb, :],
                        in0=yB[cb][:, 0:TN],
                        scalar=g2c[:, cb:cb + 1],
                        in1=rinvrep,
                        op0=ALU.mult, op1=ALU.mult,
                    )
                nc.sync.dma_start(out=ov[bl, :, :, t0:t0 + TN], in_=yout)

            # ---------------- main pipeline (software pipelined) ----------------
            xn_fulls = []
            q_sms = []
            w2Ts = [None] * BL
            for bl in range(BL):
                xn_full = persist.tile([128, 2, N], BF16, tag="xnfull")
                xn_fulls.append(xn_full)
                q_sm = persist.tile([128, 4, N], BF16, tag="qsm")
                q_sms.append(q_sm)
                ctx = ps_ctx.tile([128, 512], F32, tag="ctx")
                nc.vector.memset(ctx, 0.0)

                xts = {}
                fr = {}
                bst = {}
                for j in range(NT + 3):
                    if j < NT:
                        if bl == 0 and j in xts_pre:
                            xts[j] = xts_pre.pop(j)
                        else:
                            xts[j] = emit_load(bl, j)
                    if j >= 1 and j - 1 < NT:
                        fr[j - 1] = emit_front1(bl, j - 1, xts[j - 1])
                    if bl > 0 and j >= 1 and j - 1 in bst:
                        emit_b2(bl - 1, j - 1, *bst.pop(j - 1))
                    if j >= 2 and j - 2 < NT:
                        eq = emit_qproj(bl, j - 2, xn_full)
                        emit_qdiv(bl, j - 2, *eq, q_sm)
                    if j >= 1 and j - 1 < NT:
                        emit_front2(bl, j - 1, xts.pop(j - 1), fr.pop(j - 1), xn_full)
                    if j >= 3:
                        emit_kvpath(bl, j - 3, xn_full, ctx)
                    if bl > 0 and j < NT:
                        bst[j] = emit_b1(bl - 1, j, q_sms[bl - 1], w2Ts[bl - 1])
                if bl > 0:
                    for jj in sorted(bst):
                        emit_b2(bl - 1, jj, *bst.pop(jj))
                w2Ts[bl] = emit_epilogue(bl, ctx)
            bst = {}
            for j in range(NT + 2):
                if j >= 2:
                    emit_b2(BL - 1, j - 2, *bst.pop(j - 2))
                if j < NT:
                    bst[j] = emit_b1(BL - 1, j, q_sms[BL - 1], w2Ts[BL - 1])

    nc.finalize()
    return nc


_NC_CACHE = None


def kernel(x, g1, Wqkv, Wout, bout, g2):
    global _NC_CACHE
    x = np.ascontiguousarray(np.asarray(x, dtype=np.float32))
    g1 = np.asarray(g1, dtype=np.float32)
    Wqkv = np.ascontiguousarray(np.asarray(Wqkv, dtype=np.float32))
    Wout = np.ascontiguousarray(np.asarray(Wout, dtype=np.float32))
    bout = np.asarray(bout, dtype=np.float32)
    g2 = np.asarray(g2, dtype=np.float32)

    b, c, H, W = x.shape
    xr = x.reshape(b, c, H * W)
    if _NC_CACHE is None:
        _NC_CACHE = build_kernel()
    nc = _NC_CACHE

    in_maps = []
    for core in range(8):
        in_maps.append({
            "x": np.ascontiguousarray(xr[core * BL:(core + 1) * BL]),
            "Wqkv": Wqkv, "Wout": Wout, "bout": bout, "g1": g1, "g2": g2,
        })
    res = run_bass_kernel_spmd(nc, in_maps, core_ids=list(range(8)))
    out = np.concatenate([m["out"] for m in res.results], axis=0)
    return out.reshape(b, c, H, W).astype(np.float32)


if __name__ == "__main__":
    nc = build_kernel()
    from concourse.timeline_sim import TimelineSim
    print("sim ns:", TimelineSim(nc, trace=False).simulate())


# revision 3
# speedup vs baseline: 1.0633x; 1.0633x over previous
"""LinearAttention kernel for Trainium2, 8 NeuronCores, data-parallel over batch.

v2: fp8 DoubleRow matmuls; ACT runs only {Exp, Square, Copy} (single act table);
rsqrt/divide done on DVE via pow/divide ALU ops; per-token reductions computed
compactly ([128,4] per tile) and replicated through tiny PE outer-products.

Scaling bookkeeping (all cancel in the final rms-norm):
  Wqkv stored *16 (fp8 range), exp() applied with scale=1/16, bias=-1.5.
  q_sm stored *16 (block-diag mask folds 1/16 into the replicated qden).
  W2 stored = (Wout @ ctx^T / kden) * 0.5  (= 64 * W2_true; v carries its 16)
  => y psum = 1024 * y_true; bout row scaled *1024; rms-norm removes it all.

Per-batch layouts:
  x, xn     [c=2x128, n]    channels on partitions (xn fp8)
  q psum    [128, TN]       per ob; expq fp32r SBUF; q_sm fp8 = expq/qdrep
  kT/vT     [tok 128, 512]  per 128-token block (k and v each one DR matmul)
  ctx psum  [66, 8*64]      rows 0..63 = sum exp(k)*v, row 64 = kden
  y psum    [c 2x128, TN]   stage B in [c, n]; per-token rsqrt replicated via
                            transpose + outer-products; final scale on Pool.
"""

import numpy as np

import concourse.bass as bass
import concourse.tile as tile
from concourse import bacc, mybir
from concourse.bass_utils import run_bass_kernel_spmd
from concourse.masks import make_identity

F32 = mybir.dt.float32
F32R = mybir.dt.float32r
BF16 = mybir.dt.bfloat16
FP8 = mybir.dt.float8e4
U32 = mybir.dt.uint32

AF = mybir.ActivationFunctionType
ALU = mybir.AluOpType
DR = mybir.MatmulPerfMode.DoubleRow

B = 16          # total batches
BL = 2          # batches per core
C = 256         # in channels
HID = 512       # heads * dim_head
HEADS = 8
DH = 64         # dim head
N = 4096        # tokens (64*64)
TN = 512        # token tile
NT = N // TN    # 8 token tiles per batch
NB = TN // 128  # 4 128-token blocks per tile


def build_kernel():
    nc = bacc.Bacc("TRN2", target_bir_lowering=False, debug=False, num_devices=8)

    x_d = nc.dram_tensor("x", [BL, C, N], F32, kind="ExternalInput").ap()
    wqkv_d = nc.dram_tensor("Wqkv", [3 * HID, C], F32, kind="ExternalInput").ap()
    wout_d = nc.dram_tensor("Wout", [C, HID], F32, kind="ExternalInput").ap()
    bout_d = nc.dram_tensor("bout", [C], F32, kind="ExternalInput").ap()
    g1_d = nc.dram_tensor("g1", [C], F32, kind="ExternalInput").ap()
    g2_d = nc.dram_tensor("g2", [C], F32, kind="ExternalInput").ap()
    o_d = nc.dram_tensor("out", [BL, C, N], F32, kind="ExternalOutput").ap()

    xv = x_d.rearrange("b (cb p) n -> b p cb n", cb=2)
    ov = o_d.rearrange("b (cb p) n -> b p cb n", cb=2)

    with tile.TileContext(nc) as tc:
        with (
            tc.tile_pool(name="const", bufs=1) as const,
            tc.tile_pool(name="wt", bufs=1) as wt,
            tc.tile_pool(name="stage", bufs=1) as stage,
            tc.tile_pool(name="xin", bufs=3) as xin,
            tc.tile_pool(name="front", bufs=2) as front,
            tc.tile_pool(name="qwork", bufs=2) as qwork,
            tc.tile_pool(name="kvw", bufs=3) as kvw,
            tc.tile_pool(name="persist", bufs=2) as persist,
            tc.tile_pool(name="bwork", bufs=2) as bwork,
            tc.tile_pool(name="ps_a", bufs=4, space="PSUM") as ps_a,
            tc.tile_pool(name="ps_sm", bufs=1, space="PSUM") as ps_sm,
            tc.tile_pool(name="ps_kv", bufs=2, space="PSUM") as ps_kv,
            tc.tile_pool(name="ps_ctx", bufs=1, space="PSUM") as ps_ctx,
        ):
            # ---------------- constants ----------------
            ident = const.tile([128, 128], F32)
            make_identity(nc, ident)

            ones2_f8 = const.tile([128, 2, 1], FP8)
            nc.gpsimd.memset(ones2_f8, 1.0)
            onerow_bf = const.tile([1, 128], BF16)
            nc.gpsimd.memset(onerow_bf, 1.0)

            # block-diag [128,128] fp32r, value 1/16: replicated per-head sums
            bd_f = const.tile([128, 128], F32)
            nc.gpsimd.memset(bd_f, 0.0)
            nc.gpsimd.memset(bd_f[0:64, 0:64], 1.0 / 16.0)
            nc.gpsimd.memset(bd_f[64:128, 64:128], 1.0 / 16.0)
            bdr = const.tile([128, 128], BF16)
            nc.vector.tensor_copy(out=bdr, in_=bd_f)

            onecol_bf = const.tile([128, 1], BF16)
            nc.gpsimd.memset(onecol_bf, 1.0)
            onesrow = const.tile([1, 512], BF16)
            nc.gpsimd.memset(onesrow, 1.0)
            biasm = const.tile([128, 1], F32)
            nc.gpsimd.memset(biasm, -1.5)

            g1c = const.tile([128, 2], F32)
            nc.sync.dma_start(out=g1c, in_=g1_d.rearrange("(cb p) -> p cb", cb=2))
            g1s = const.tile([128, 2], F32)
            nc.vector.tensor_scalar_mul(out=g1s, in0=g1c, scalar1=16.0)
            g2c = const.tile([128, 2], F32)
            nc.sync.dma_start(out=g2c, in_=g2_d.rearrange("(cb p) -> p cb", cb=2))

            # bout as a row, scaled by 1024 (total y scale), bf16
            brow = const.tile([1, 256], F32)
            nc.sync.dma_start(out=brow, in_=bout_d.rearrange("(a c) -> a c", a=1))
            broww = const.tile([1, 256], BF16)
            nc.vector.tensor_scalar_mul(out=broww, in0=brow, scalar1=1024.0)

            # prefetch the first x tiles before the weight-prep flood
            xts_pre = {}
            for jpre in range(2):
                xt_pre = xin.tile([128, 2, TN], F32, tag="xt", name="xt_pre")
                nc.sync.dma_start(out=xt_pre, in_=xv[0, :, :, jpre * TN:(jpre + 1) * TN])
                xts_pre[jpre] = xt_pre

            # ---------------- weights ----------------
            # Wqkv [1536, 256] -> wqkvT fp8 [c(2x128), cb, 1536], rows *g1*16
            wq_nat = stage.tile([128, 12, 256], F32, tag="wnat")
            nc.sync.dma_start(
                out=wq_nat, in_=wqkv_d.rearrange("(ob p) c -> p ob c", p=128)
            )
            wqkvTb = wt.tile([128, 2, 1536], BF16)
            for ob in range(12):
                for cb in range(2):
                    pt = ps_a.tile([128, 512], F32, tag="pa", name="pt")
                    nc.tensor.transpose(
                        pt[:, 0:128], wq_nat[:, ob, cb * 128:(cb + 1) * 128], ident
                    )
                    nc.vector.tensor_scalar_mul(
                        out=wqkvTb[:, cb, ob * 128:(ob + 1) * 128],
                        in0=pt[:, 0:128],
                        scalar1=g1s[:, cb:cb + 1],
                    )
            # Wout [256, 512] -> woutT [e=64, h, 256] fp32r
            wo_nat = stage.tile([128, 2, 512], F32, tag="wnat")
            nc.sync.dma_start(
                out=wo_nat, in_=wout_d.rearrange("(ob p) h -> p ob h", p=128)
            )
            woutT = wt.tile([64, 8, 256], F32R)
            for h in range(HEADS):
                for ob in range(2):
                    pt = ps_a.tile([128, 512], F32, tag="pa", name="pt")
                    nc.tensor.transpose(
                        pt[0:64, 0:128], wo_nat[:, ob, h * 64:(h + 1) * 64], ident
                    )
                    nc.vector.tensor_copy(
                        out=woutT[:, h, ob * 128:(ob + 1) * 128], in_=pt[0:64, 0:128]
                    )
            scl_f = const.tile([1, 2], F32)
            nc.gpsimd.memset(scl_f, 0.5)
            scl_r = const.tile([1, 2], F32R)
            nc.vector.tensor_copy(out=scl_r, in_=scl_f)  # kden transpose helper

            # ---------------- helpers ----------------
            MAGIC = 0x5F3759DF + 0x02000000  # rsqrt seed for m = ssq/256
            MAGIC_SUB = 0x7FFFFFFF - MAGIC  # overflow-free: C-(i>>1) = ((i>>1)^0x7fffffff) - this

            def rsqrt_compact(cT, sm, pool, tag):
                """cT [128,4] f32 psum of per-token ssq -> replicated
                [128, 512] f32 psum with 16/sqrt(ssq) (bit-hack + 1 Newton).
                sm is the host psum tile for the transpose scratch."""
                t1 = pool.tile([128, 4], U32, tag=tag + "t1", name="t1")
                nc.vector.tensor_scalar(
                    out=t1, in0=cT.bitcast(U32), scalar1=1, scalar2=0x7FFFFFFF,
                    op0=ALU.logical_shift_right, op1=ALU.bitwise_xor,
                )
                y0 = pool.tile([128, 4], U32, tag=tag + "y0", name="y0")
                nc.vector.tensor_scalar(
                    out=y0, in0=t1, scalar1=MAGIC_SUB, scalar2=None, op0=ALU.subtract
                )
                y0f = y0.bitcast(F32)
                t2 = pool.tile([128, 4], F32, tag=tag + "t2", name="t2")
                nc.vector.tensor_mul(t2, y0f, y0f)
                nc.vector.tensor_mul(t2, t2, cT)
                t3 = pool.tile([128, 4], F32, tag=tag + "t3", name="t3")
                nc.vector.tensor_scalar(
                    out=t3, in0=t2, scalar1=-1.0 / 512.0, scalar2=1.5,
                    op0=ALU.mult, op1=ALU.add,
                )
                y1 = pool.tile([128, 4], F32, tag=tag + "y1", name="y1")
                nc.vector.tensor_mul(y1, y0f, t3)
                nc.vector.tensor_mul(t2, y1, y1)
                nc.vector.tensor_mul(t2, t2, cT)
                nc.vector.tensor_scalar(
                    out=t3, in0=t2, scalar1=-1.0 / 512.0, scalar2=1.5,
                    op0=ALU.mult, op1=ALU.add,
                )
                vT = pool.tile([128, 4], F32, tag=tag + "vT", name="vT")
                nc.vector.tensor_mul(vT, y1, t3)
                # replicate: transpose -> sbuf bf16 -> row-flatten -> outer
                tt = sm[0:4, 128:256]
                nc.tensor.transpose(tt, vT, ident)
                v_sb = pool.tile([4, 128], BF16, tag=tag + "sb", name="v_sb")
                nc.vector.tensor_copy(out=v_sb, in_=tt)
                vrow = pool.tile([1, 512], BF16, tag=tag + "row", name="vrow")
                nc.sync.dma_start(out=vrow, in_=v_sb)
                nc.tensor.matmul(sm, onerow_bf, vrow, start=True, stop=True)
                return sm

            def emit_load(bl, j):
                """Prefetch x tile j."""
                xt = xin.tile([128, 2, TN], F32, tag="xt")
                nc.sync.dma_start(out=xt, in_=xv[bl, :, :, j * TN:(j + 1) * TN])
                return xt

            def emit_front1(bl, j, xt):
                """x^2 + compact per-token ssq for tile j."""
                x2 = front.tile([128, 2, TN], FP8, tag="x2")
                nc.gpsimd.tensor_mul(x2, xt, xt)
                sm = ps_sm.tile([128, 512], F32, tag="sm", name="sm")
                for nb in range(NB):
                    nc.tensor.matmul(
                        sm[:, nb:nb + 1],
                        x2[:, :, nb * 128:(nb + 1) * 128],
                        ones2_f8,
                        start=True, stop=True,
                        perf_mode=DR,
                        skip_group_check=True,
                    )
                return sm

            def emit_front2(bl, j, xt, sm, xn_full):
                """rsqrt + normalize tile j into xn_full (fp8)."""
                t0 = j * TN
                sinvrep = rsqrt_compact(sm[:, 0:4], sm, front, "sinv")
                for cb in range(2):
                    nc.vector.tensor_mul(
                        xn_full[:, cb, t0:t0 + TN], xt[:, cb, :], sinvrep[:, 0:TN]
                    )

            def emit_qproj(bl, j, xn_full):
                """S1: q projection + exp for tile j -> (expq, pq psum tiles)."""
                t0 = j * TN
                expq = qwork.tile([128, 4, TN], BF16, tag="expq")
                pqs = []
                for ob in range(4):
                    pq = ps_a.tile([128, 512], F32, tag="pa", name="pq")
                    pqs.append(pq)
                    for cb in range(2):
                        nc.tensor.matmul(
                            pq[:, 0:TN],
                            wqkvTb[:, cb, ob * 128:(ob + 1) * 128],
                            xn_full[:, cb, t0:t0 + TN],
                            start=(cb == 0), stop=(cb == 1),
                        )
                    nc.scalar.activation(
                        out=expq[:, ob, :], in_=pq[:, 0:TN],
                        func=AF.Exp, scale=1.0 / 16.0, bias=biasm,
                    )
                return expq, pqs

            def emit_qdiv(bl, j, expq, pqs, q_sm):
                """S2: replicated per-head denominators (into the freed pq
                banks), reciprocal to SBUF, multiply on Pool."""
                t0 = j * TN
                for ob in range(4):
                    nc.tensor.matmul(
                        pqs[ob][:, 0:TN], bdr, expq[:, ob, :],
                        start=True, stop=True,
                    )
                    qdinv = qwork.tile([128, TN], F32R, tag="qdinv")
                    with nc.allow_low_precision(reason="q softmax denom"):
                        nc.vector.reciprocal(out=qdinv, in_=pqs[ob][:, 0:TN])
                    nc.gpsimd.tensor_mul(
                        q_sm[:, ob, t0:t0 + TN], expq[:, ob, :], qdinv
                    )

            def emit_kvpath(bl, j, xn_full, ctx):
                """kv projection + exp(k) + ctx accumulation for tile j."""
                t0 = j * TN
                for half in range(2):  # two 2-block pairs
                    expk2 = kvw.tile([128, 2, 8, 64], BF16, tag="expk")
                    vt2 = kvw.tile([128, 2, 8, 66], BF16, tag="vt")
                    nc.gpsimd.memset(vt2[:, :, :, 64:65], 1.0)
                    nc.gpsimd.memset(vt2[:, :, :, 65:66], 0.0)
                    for bi in range(2):
                        nb = half * 2 + bi
                        pk = ps_kv.tile([128, 512], F32, tag="pkv", name="pk")
                        for cb in range(2):
                            nc.tensor.matmul(
                                pk,
                                xn_full[:, cb, t0 + nb * 128:t0 + (nb + 1) * 128],
                                wqkvTb[:, cb, 512:1024],
                                start=(cb == 0), stop=(cb == 1),
                            )
                        pv = ps_kv.tile([128, 512], F32, tag="pkv", name="pv")
                        for cb in range(2):
                            nc.tensor.matmul(
                                pv,
                                xn_full[:, cb, t0 + nb * 128:t0 + (nb + 1) * 128],
                                wqkvTb[:, cb, 1024:1536],
                                start=(cb == 0), stop=(cb == 1),
                            )
                        nc.scalar.activation(
                            out=expk2[:, bi], in_=pk,
                            func=AF.Exp, scale=1.0 / 16.0, bias=biasm,
                        )
                        if half == 0:
                            nc.scalar.copy(
                                out=vt2[:, bi, :, 0:64],
                                in_=pv.rearrange("p (h e) -> p h e", h=8),
                            )
                        else:
                            nc.vector.tensor_copy(
                                out=vt2[:, bi, :, 0:64],
                                in_=pv.rearrange("p (h e) -> p h e", h=8),
                            )
                    gpair = j * 2 + half
                    for bi in range(2):
                        for h in range(HEADS):
                            nc.tensor.matmul(
                                ctx[0:66, h * 64:(h + 1) * 64],
                                vt2[:, bi, h, :],
                                expk2[:, bi, h, :],
                                start=False,
                                stop=(gpair == 2 * NT - 1 and bi == 1),
                                skip_group_check=True,
                            )

            def emit_epilogue(bl, ctx):
                """Build w2T fp8 [128, 4, 256] = (Wout_h @ ctx_h^T / kden * .5)^T"""
                ctx_sb = persist.tile([64, 512], F32R, tag="ctxsb")
                nc.vector.tensor_copy(out=ctx_sb, in_=ctx[0:64, :])
                kdrow = persist.tile([1, 512], F32R, tag="kdrow")
                with nc.allow_low_precision(reason="k softmax denominators"):
                    nc.vector.reciprocal(out=kdrow, in_=ctx[64:65, :])
                # transpose kden row -> columns [64, 8]
                pkd = ps_a.tile([128, 512], F32, tag="pa", name="pkd")
                for h in range(HEADS):
                    nc.tensor.matmul(
                        pkd[0:64, 2 * h:2 * h + 2],
                        kdrow[0:1, h * 64:(h + 1) * 64],
                        scl_r,
                        start=True, stop=True,
                        skip_group_check=True,
                    )
                kdcol = persist.tile([64, 8, 1], F32, tag="kdcol")
                pkd_v = pkd[0:64, 0:16].rearrange("p (h t) -> p h t", t=2)
                nc.vector.tensor_copy(out=kdcol, in_=pkd_v[:, :, 0:1])
                w2stg = persist.tile([64, 8, 256], BF16, tag="w2stg")
                for h in range(HEADS):
                    pw2 = ps_a.tile([128, 512], F32, tag="pa", name="pw2")
                    nc.tensor.matmul(
                        pw2[0:64, 0:256],
                        ctx_sb[:, h * 64:(h + 1) * 64],
                        woutT[:, h, :],
                        start=True, stop=True,
                    )
                    nc.vector.tensor_scalar_mul(
                        out=w2stg[:, h, :],
                        in0=pw2[0:64, 0:256],
                        scalar1=kdcol[:, h, :],
                    )
                w2T = persist.tile([128, 4, 256], BF16, tag="w2T")
                for h in range(HEADS):
                    nc.sync.dma_start(
                        out=w2T[(h % 2) * 64:(h % 2) * 64 + 64, h // 2, :],
                        in_=w2stg[:, h, :],
                    )
                return w2T

            def emit_b1(bl, j, q_sm, w2T):
                """Stage B matmuls + y^2 for tile j."""
                t0 = j * TN
                yB = [None, None]
                y2 = bwork.tile([128, 2, TN], BF16, tag="y2")
                for cb in range(2):
                    yB[cb] = ps_a.tile([128, 512], F32, tag="pa", name="yB")
                    for cp in range(4):
                        nc.tensor.matmul(
                            yB[cb][:, 0:TN],
                            w2T[:, cp, cb * 128:(cb + 1) * 128],
                            q_sm[:, cp, t0:t0 + TN],
                            start=(cp == 0), stop=False,
                            skip_group_check=True,
                        )
                    nc.tensor.matmul(
                        yB[cb][:, 0:TN],
                        broww[0:1, cb * 128:(cb + 1) * 128],
                        onesrow,
                        start=False, stop=True,
                        skip_group_check=True,
                    )
                    nc.scalar.activation(
                        out=y2[:, cb, :], in_=yB[cb][:, 0:TN], func=AF.Square
                    )
                smb = ps_sm.tile([128, 512], F32, tag="sm", name="smb")
                for nb in range(NB):
                    for cb in range(2):
                        nc.tensor.matmul(
                            smb[:, nb:nb + 1],
                            y2[:, cb, nb * 128:(nb + 1) * 128],
                            onecol_bf,
                            start=(cb == 0), stop=(cb == 1),
                            skip_group_check=True,
                        )
                return yB, smb

            def emit_b2(bl, j, yB, smb):
                """Stage B rms-out + store for tile j."""
                t0 = j * TN
                rrep_ps = rsqrt_compact(smb[:, 0:4], smb, bwork, "rinv")
                rinvrep = bwork.tile([128, TN], F32, tag="rinvsb")
                nc.scalar.copy(out=rinvrep, in_=rrep_ps[:, 0:TN])
                yout = bwork.tile([128, 2, TN], F32, tag="yout")
                for cb in range(2):
                    nc.vector.scalar_tensor_tensor(
                        out=yout[:, cb, :],
                        in0=yB[cb][:, 0:TN],
                        scalar=g2c[:, cb:cb + 1],
                        in1=rinvrep,
                        op0=ALU.mult, op1=ALU.mult,
                    )
                nc.sync.dma_start(out=ov[bl, :, :, t0:t0 + TN], in_=yout)

            # ---------------- main pipeline (software pipelined) ----------------
            xn_fulls = []
            ctxs = {}
            q_sms = []
            w2Ts = [None] * BL
            for bl in range(BL):
                xn_full = persist.tile([128, 2, N], BF16, tag="xnfull")
                xn_fulls.append(xn_full)
                q_sm = persist.tile([128, 4, N], BF16, tag="qsm")
                q_sms.append(q_sm)
                ctx = ps_ctx.tile([128, 512], F32, tag="ctx")
                nc.vector.memset(ctx, 0.0)
                ctxs[bl] = ctx

                xts = {}
                fr = {}
                bst = {}
                for j in range(NT + 3):
                    # overlap the previous batch's epilogue with our warmup
                    if bl > 0 and j == 2:
                        w2Ts[bl - 1] = emit_epilogue(bl - 1, ctxs[bl - 1])
                    if j < NT:
                        if bl == 0 and j in xts_pre:
                            xts[j] = xts_pre.pop(j)
                        else:
                            xts[j] = emit_load(bl, j)
                    if j >= 1 and j - 1 < NT:
                        fr[j - 1] = emit_front1(bl, j - 1, xts[j - 1])
                    if bl > 0 and j >= 3 and j - 3 in bst:
                        emit_b2(bl - 1, j - 3, *bst.pop(j - 3))
                    if j >= 2 and j - 2 < NT:
                        eq = emit_qproj(bl, j - 2, xn_full)
                        emit_qdiv(bl, j - 2, *eq, q_sm)
                    if j >= 1 and j - 1 < NT:
                        emit_front2(bl, j - 1, xts.pop(j - 1), fr.pop(j - 1), xn_full)
                    if j >= 3:
                        emit_kvpath(bl, j - 3, xn_full, ctx)
                    if bl > 0 and 2 <= j and j - 2 < NT:
                        bst[j - 2] = emit_b1(bl - 1, j - 2, q_sms[bl - 1],
                                             w2Ts[bl - 1])
                if bl > 0:
                    for jj in sorted(bst):
                        emit_b2(bl - 1, jj, *bst.pop(jj))
                if bl == BL - 1:
                    w2Ts[bl] = emit_epilogue(bl, ctx)
            bst = {}
            for j in range(NT + 2):
                if j >= 2:
                    emit_b2(BL - 1, j - 2, *bst.pop(j - 2))
                if j < NT:
                    bst[j] = emit_b1(BL - 1, j, q_sms[BL - 1], w2Ts[BL - 1])

    nc.finalize()
    return nc


_NC_CACHE = None


def kernel(x, g1, Wqkv, Wout, bout, g2):
    global _NC_CACHE
    x = np.ascontiguousarray(np.asarray(x, dtype=np.float32))
    g1 = np.asarray(g1, dtype=np.float32)
    Wqkv = np.ascontiguousarray(np.asarray(Wqkv, dtype=np.float32))
    Wout = np.ascontiguousarray(np.asarray(Wout, dtype=np.float32))
    bout = np.asarray(bout, dtype=np.float32)
    g2 = np.asarray(g2, dtype=np.float32)

    b, c, H, W = x.shape
    xr = x.reshape(b, c, H * W)
    if _NC_CACHE is None:
        _NC_CACHE = build_kernel()
    nc = _NC_CACHE

    in_maps = []
    for core in range(8):
        in_maps.append({
            "x": np.ascontiguousarray(xr[core * BL:(core + 1) * BL]),
            "Wqkv": Wqkv, "Wout": Wout, "bout": bout, "g1": g1, "g2": g2,
        })
    res = run_bass_kernel_spmd(nc, in_maps, core_ids=list(range(8)))
    out = np.concatenate([m["out"] for m in res.results], axis=0)
    return out.reshape(b, c, H, W).astype(np.float32)


if __name__ == "__main__":
    nc = build_kernel()
    from concourse.timeline_sim import TimelineSim
    print("sim ns:", TimelineSim(nc, trace=False).simulate())


# revision 4
# speedup vs baseline: 1.1434x; 1.0753x over previous
"""LinearAttention kernel for Trainium2, 8 NeuronCores, data-parallel over batch.

v2: fp8 DoubleRow matmuls; ACT runs only {Exp, Square, Copy} (single act table);
rsqrt/divide done on DVE via pow/divide ALU ops; per-token reductions computed
compactly ([128,4] per tile) and replicated through tiny PE outer-products.

Scaling bookkeeping (all cancel in the final rms-norm):
  Wqkv stored *16 (fp8 range), exp() applied with scale=1/16, bias=-1.5.
  q_sm stored *16 (block-diag mask folds 1/16 into the replicated qden).
  W2 stored = (Wout @ ctx^T / kden) * 0.5  (= 64 * W2_true; v carries its 16)
  => y psum = 1024 * y_true; bout row scaled *1024; rms-norm removes it all.

Per-batch layouts:
  x, xn     [c=2x128, n]    channels on partitions (xn fp8)
  q psum    [128, TN]       per ob; expq fp32r SBUF; q_sm fp8 = expq/qdrep
  kT/vT     [tok 128, 512]  per 128-token block (k and v each one DR matmul)
  ctx psum  [66, 8*64]      rows 0..63 = sum exp(k)*v, row 64 = kden
  y psum    [c 2x128, TN]   stage B in [c, n]; per-token rsqrt replicated via
                            transpose + outer-products; final scale on Pool.
"""

import numpy as np

import concourse.bass as bass
import concourse.tile as tile
from concourse import bacc, mybir
from concourse.bass_utils import run_bass_kernel_spmd
from concourse.masks import make_identity

F32 = mybir.dt.float32
F32R = mybir.dt.float32r
BF16 = mybir.dt.bfloat16
FP8 = mybir.dt.float8e4
U32 = mybir.dt.uint32

AF = mybir.ActivationFunctionType
ALU = mybir.AluOpType
DR = mybir.MatmulPerfMode.DoubleRow

B = 16          # total batches
BL = 2          # batches per core
C = 256         # in channels
HID = 512       # heads * dim_head
HEADS = 8
DH = 64         # dim head
N = 4096        # tokens (64*64)
TN = 512        # token tile
NT = N // TN    # 8 token tiles per batch
NB = TN // 128  # 4 128-token blocks per tile


def build_kernel():
    nc = bacc.Bacc("TRN2", target_bir_lowering=False, debug=False, num_devices=8)

    x_d = nc.dram_tensor("x", [BL, C, N], F32, kind="ExternalInput").ap()
    wqkv_d = nc.dram_tensor("Wqkv", [3 * HID, C], F32, kind="ExternalInput").ap()
    wout_d = nc.dram_tensor("Wout", [C, HID], F32, kind="ExternalInput").ap()
    bout_d = nc.dram_tensor("bout", [C], F32, kind="ExternalInput").ap()
    g1_d = nc.dram_tensor("g1", [C], F32, kind="ExternalInput").ap()
    g2_d = nc.dram_tensor("g2", [C], F32, kind="ExternalInput").ap()
    o_d = nc.dram_tensor("out", [BL, C, N], F32, kind="ExternalOutput").ap()

    xv = x_d.rearrange("b (cb p) n -> b p cb n", cb=2)
    ov = o_d.rearrange("b (cb p) n -> b p cb n", cb=2)

    with tile.TileContext(nc) as tc:
        with (
            tc.tile_pool(name="const", bufs=1) as const,
            tc.tile_pool(name="wt", bufs=1) as wt,
            tc.tile_pool(name="stage", bufs=1) as stage,
            tc.tile_pool(name="xin", bufs=3) as xin,
            tc.tile_pool(name="front", bufs=2) as front,
            tc.tile_pool(name="qwork", bufs=2) as qwork,
            tc.tile_pool(name="kvw", bufs=3) as kvw,
            tc.tile_pool(name="persist", bufs=2) as persist,
            tc.tile_pool(name="bwork", bufs=2) as bwork,
            tc.tile_pool(name="ps_a", bufs=4, space="PSUM") as ps_a,
            tc.tile_pool(name="ps_sm", bufs=1, space="PSUM") as ps_sm,
            tc.tile_pool(name="ps_kv", bufs=2, space="PSUM") as ps_kv,
            tc.tile_pool(name="ps_ctx", bufs=1, space="PSUM") as ps_ctx,
        ):
            # ---------------- constants ----------------
            ident = const.tile([128, 128], F32)
            make_identity(nc, ident)

            ones2_f8 = const.tile([128, 2, 1], FP8)
            nc.gpsimd.memset(ones2_f8, 1.0)
            onerow_bf = const.tile([1, 128], BF16)
            nc.gpsimd.memset(onerow_bf, 1.0)

            # block-diag [128,128] fp32r, value 1/16: replicated per-head sums
            bd_f = const.tile([128, 128], F32)
            nc.gpsimd.memset(bd_f, 0.0)
            nc.gpsimd.memset(bd_f[0:64, 0:64], 1.0 / 16.0)
            nc.gpsimd.memset(bd_f[64:128, 64:128], 1.0 / 16.0)
            bdr = const.tile([128, 128], BF16)
            nc.vector.tensor_copy(out=bdr, in_=bd_f)

            onecol_bf = const.tile([128, 1], BF16)
            nc.gpsimd.memset(onecol_bf, 1.0)
            onesrow = const.tile([1, 512], BF16)
            nc.gpsimd.memset(onesrow, 1.0)
            biasm = const.tile([128, 1], F32)
            nc.gpsimd.memset(biasm, -1.5)

            g1c = const.tile([128, 2], F32)
            nc.sync.dma_start(out=g1c, in_=g1_d.rearrange("(cb p) -> p cb", cb=2))
            g1s = const.tile([128, 2], F32)
            nc.vector.tensor_scalar_mul(out=g1s, in0=g1c, scalar1=16.0)
            g2c = const.tile([128, 2], F32)
            nc.sync.dma_start(out=g2c, in_=g2_d.rearrange("(cb p) -> p cb", cb=2))

            # bout as a row, scaled by 1024 (total y scale), bf16
            brow = const.tile([1, 256], F32)
            nc.sync.dma_start(out=brow, in_=bout_d.rearrange("(a c) -> a c", a=1))
            broww = const.tile([1, 256], BF16)
            nc.vector.tensor_scalar_mul(out=broww, in0=brow, scalar1=1024.0)

            # prefetch the first x tiles before the weight-prep flood
            xts_pre = {}
            for jpre in range(2):
                xt_pre = xin.tile([128, 2, TN], F32, tag="xt", name="xt_pre")
                nc.sync.dma_start(out=xt_pre, in_=xv[0, :, :, jpre * TN:(jpre + 1) * TN])
                xts_pre[jpre] = xt_pre

            # ---------------- weights ----------------
            # Wqkv [1536, 256] -> wqkvT fp8 [c(2x128), cb, 1536], rows *g1*16
            wq_nat = stage.tile([128, 12, 256], F32, tag="wnat")
            nc.sync.dma_start(
                out=wq_nat, in_=wqkv_d.rearrange("(ob p) c -> p ob c", p=128)
            )
            wqkvTb = wt.tile([128, 2, 1536], BF16)
            for ob in range(12):
                for cb in range(2):
                    pt = ps_a.tile([128, 512], F32, tag="pa", name="pt")
                    nc.tensor.transpose(
                        pt[:, 0:128], wq_nat[:, ob, cb * 128:(cb + 1) * 128], ident
                    )
                    nc.vector.tensor_scalar_mul(
                        out=wqkvTb[:, cb, ob * 128:(ob + 1) * 128],
                        in0=pt[:, 0:128],
                        scalar1=g1s[:, cb:cb + 1],
                    )
            # Wout [256, 512] -> woutT [e=64, h, 256] fp32r
            wo_nat = stage.tile([128, 2, 512], F32, tag="wnat")
            nc.sync.dma_start(
                out=wo_nat, in_=wout_d.rearrange("(ob p) h -> p ob h", p=128)
            )
            woutT = wt.tile([64, 8, 256], F32R)
            for h in range(HEADS):
                for ob in range(2):
                    pt = ps_a.tile([128, 512], F32, tag="pa", name="pt")
                    nc.tensor.transpose(
                        pt[0:64, 0:128], wo_nat[:, ob, h * 64:(h + 1) * 64], ident
                    )
                    nc.vector.tensor_copy(
                        out=woutT[:, h, ob * 128:(ob + 1) * 128], in_=pt[0:64, 0:128]
                    )
            scl_f = const.tile([1, 2], F32)
            nc.gpsimd.memset(scl_f, 0.5)
            scl_r = const.tile([1, 2], F32R)
            nc.vector.tensor_copy(out=scl_r, in_=scl_f)  # kden transpose helper

            # ---------------- helpers ----------------
            MAGIC = 0x5F3759DF + 0x02000000  # rsqrt seed for m = ssq/256
            MAGIC_SUB = 0x7FFFFFFF - MAGIC  # overflow-free: C-(i>>1) = ((i>>1)^0x7fffffff) - this

            def rsqrt_compact(cT, sm, pool, tag):
                """cT [128,4] f32 psum of per-token ssq -> replicated
                [128, 512] f32 psum with 16/sqrt(ssq) (bit-hack + 1 Newton).
                sm is the host psum tile for the transpose scratch."""
                t1 = pool.tile([128, 4], U32, tag=tag + "t1", name="t1")
                nc.vector.tensor_scalar(
                    out=t1, in0=cT.bitcast(U32), scalar1=1, scalar2=0x7FFFFFFF,
                    op0=ALU.logical_shift_right, op1=ALU.bitwise_xor,
                )
                y0 = pool.tile([128, 4], U32, tag=tag + "y0", name="y0")
                nc.vector.tensor_scalar(
                    out=y0, in0=t1, scalar1=MAGIC_SUB, scalar2=None, op0=ALU.subtract
                )
                y0f = y0.bitcast(F32)
                t2 = pool.tile([128, 4], F32, tag=tag + "t2", name="t2")
                nc.vector.tensor_mul(t2, y0f, y0f)
                nc.vector.tensor_mul(t2, t2, cT)
                t3 = pool.tile([128, 4], F32, tag=tag + "t3", name="t3")
                nc.vector.tensor_scalar(
                    out=t3, in0=t2, scalar1=-1.0 / 512.0, scalar2=1.5,
                    op0=ALU.mult, op1=ALU.add,
                )
                y1 = pool.tile([128, 4], F32, tag=tag + "y1", name="y1")
                nc.vector.tensor_mul(y1, y0f, t3)
                nc.vector.tensor_mul(t2, y1, y1)
                nc.vector.tensor_mul(t2, t2, cT)
                nc.vector.tensor_scalar(
                    out=t3, in0=t2, scalar1=-1.0 / 512.0, scalar2=1.5,
                    op0=ALU.mult, op1=ALU.add,
                )
                vT = pool.tile([128, 4], F32, tag=tag + "vT", name="vT")
                nc.vector.tensor_mul(vT, y1, t3)
                # replicate: transpose -> sbuf bf16 -> row-flatten -> outer
                tt = sm[0:4, 128:256]
                nc.tensor.transpose(tt, vT, ident)
                v_sb = pool.tile([4, 128], BF16, tag=tag + "sb", name="v_sb")
                nc.vector.tensor_copy(out=v_sb, in_=tt)
                vrow = pool.tile([1, 512], BF16, tag=tag + "row", name="vrow")
                nc.sync.dma_start(out=vrow, in_=v_sb)
                nc.tensor.matmul(sm, onerow_bf, vrow, start=True, stop=True)
                return sm

            def emit_load(bl, j):
                """Prefetch x tile j."""
                xt = xin.tile([128, 2, TN], F32, tag="xt")
                nc.sync.dma_start(out=xt, in_=xv[bl, :, :, j * TN:(j + 1) * TN])
                return xt

            def emit_front1(bl, j, xt):
                """x^2 + compact per-token ssq for tile j."""
                x2 = front.tile([128, 2, TN], FP8, tag="x2")
                nc.gpsimd.tensor_mul(x2, xt, xt)
                sm = ps_sm.tile([128, 512], F32, tag="sm", name="sm")
                for nb in range(NB):
                    nc.tensor.matmul(
                        sm[:, nb:nb + 1],
                        x2[:, :, nb * 128:(nb + 1) * 128],
                        ones2_f8,
                        start=True, stop=True,
                        perf_mode=DR,
                        skip_group_check=True,
                    )
                return sm

            def emit_front2(bl, j, xt, sm, xn_full):
                """rsqrt + normalize tile j into xn_full (fp8)."""
                t0 = j * TN
                sinvrep = rsqrt_compact(sm[:, 0:4], sm, front, "sinv")
                for cb in range(2):
                    nc.vector.tensor_mul(
                        xn_full[:, cb, t0:t0 + TN], xt[:, cb, :], sinvrep[:, 0:TN]
                    )

            def emit_qproj(bl, j, xn_full):
                """S1: q projection + exp for tile j -> (expq, pq psum tiles)."""
                t0 = j * TN
                expq = qwork.tile([128, 4, TN], BF16, tag="expq")
                pqs = []
                for ob in range(4):
                    pq = ps_a.tile([128, 512], F32, tag="pa", name="pq")
                    pqs.append(pq)
                    for cb in range(2):
                        nc.tensor.matmul(
                            pq[:, 0:TN],
                            wqkvTb[:, cb, ob * 128:(ob + 1) * 128],
                            xn_full[:, cb, t0:t0 + TN],
                            start=(cb == 0), stop=(cb == 1),
                        )
                    nc.scalar.activation(
                        out=expq[:, ob, :], in_=pq[:, 0:TN],
                        func=AF.Exp, scale=1.0 / 16.0, bias=biasm,
                    )
                return expq, pqs

            def emit_qdiv(bl, j, expq, pqs, q_sm):
                """S2: replicated per-head denominators (into the freed pq
                banks), reciprocal to SBUF, multiply on Pool."""
                t0 = j * TN
                for ob in range(4):
                    nc.tensor.matmul(
                        pqs[ob][:, 0:TN], bdr, expq[:, ob, :],
                        start=True, stop=True,
                    )
                    qdinv = qwork.tile([128, TN], F32R, tag="qdinv")
                    with nc.allow_low_precision(reason="q softmax denom"):
                        nc.vector.reciprocal(out=qdinv, in_=pqs[ob][:, 0:TN])
                    nc.gpsimd.tensor_mul(
                        q_sm[:, ob, t0:t0 + TN], expq[:, ob, :], qdinv
                    )

            def emit_kvpath(bl, j, xn_full, ctx):
                """kv projection + exp(k) + ctx accumulation for tile j."""
                t0 = j * TN
                for half in range(2):  # two 2-block pairs
                    expk2 = kvw.tile([128, 2, 8, 64], BF16, tag="expk")
                    vt2 = kvw.tile([128, 2, 8, 66], BF16, tag="vt")
                    nc.gpsimd.memset(vt2[:, :, :, 64:65], 1.0)
                    nc.gpsimd.memset(vt2[:, :, :, 65:66], 0.0)
                    for bi in range(2):
                        nb = half * 2 + bi
                        pk = ps_kv.tile([128, 512], F32, tag="pkv", name="pk")
                        for cb in range(2):
                            nc.tensor.matmul(
                                pk,
                                xn_full[:, cb, t0 + nb * 128:t0 + (nb + 1) * 128],
                                wqkvTb[:, cb, 512:1024],
                                start=(cb == 0), stop=(cb == 1),
                            )
                        pv = ps_kv.tile([128, 512], F32, tag="pkv", name="pv")
                        for cb in range(2):
                            nc.tensor.matmul(
                                pv,
                                xn_full[:, cb, t0 + nb * 128:t0 + (nb + 1) * 128],
                                wqkvTb[:, cb, 1024:1536],
                                start=(cb == 0), stop=(cb == 1),
                            )
                        nc.scalar.activation(
                            out=expk2[:, bi], in_=pk,
                            func=AF.Exp, scale=1.0 / 16.0, bias=biasm,
                        )
                        nc.scalar.copy(
                            out=vt2[:, bi, :, 0:64],
                            in_=pv.rearrange("p (h e) -> p h e", h=8),
                        )
                    gpair = j * 2 + half
                    for bi in range(2):
                        for h in range(HEADS):
                            nc.tensor.matmul(
                                ctx[0:66, h * 64:(h + 1) * 64],
                                vt2[:, bi, h, :],
                                expk2[:, bi, h, :],
                                start=False,
                                stop=(gpair == 2 * NT - 1 and bi == 1),
                                skip_group_check=True,
                            )

            def emit_epilogue(bl, ctx):
                """Build w2T fp8 [128, 4, 256] = (Wout_h @ ctx_h^T / kden * .5)^T"""
                ctx_sb = persist.tile([64, 512], F32R, tag="ctxsb")
                nc.vector.tensor_copy(out=ctx_sb, in_=ctx[0:64, :])
                kdrow = persist.tile([1, 512], F32R, tag="kdrow")
                with nc.allow_low_precision(reason="k softmax denominators"):
                    nc.vector.reciprocal(out=kdrow, in_=ctx[64:65, :])
                # transpose kden row -> columns [64, 8]
                pkd = ps_a.tile([128, 512], F32, tag="pa", name="pkd")
                for h in range(HEADS):
                    nc.tensor.matmul(
                        pkd[0:64, 2 * h:2 * h + 2],
                        kdrow[0:1, h * 64:(h + 1) * 64],
                        scl_r,
                        start=True, stop=True,
                        skip_group_check=True,
                    )
                kdcol = persist.tile([64, 8, 1], F32, tag="kdcol")
                pkd_v = pkd[0:64, 0:16].rearrange("p (h t) -> p h t", t=2)
                nc.vector.tensor_copy(out=kdcol, in_=pkd_v[:, :, 0:1])
                w2stg = persist.tile([64, 8, 256], BF16, tag="w2stg")
                for h in range(HEADS):
                    pw2 = ps_a.tile([128, 512], F32, tag="pa", name="pw2")
                    nc.tensor.matmul(
                        pw2[0:64, 0:256],
                        ctx_sb[:, h * 64:(h + 1) * 64],
                        woutT[:, h, :],
                        start=True, stop=True,
                    )
                    nc.vector.tensor_scalar_mul(
                        out=w2stg[:, h, :],
                        in0=pw2[0:64, 0:256],
                        scalar1=kdcol[:, h, :],
                    )
                w2T = persist.tile([128, 4, 256], BF16, tag="w2T")
                for h in range(HEADS):
                    nc.sync.dma_start(
                        out=w2T[(h % 2) * 64:(h % 2) * 64 + 64, h // 2, :],
                        in_=w2stg[:, h, :],
                    )
                return w2T

            def emit_b1(bl, j, q_sm, w2T):
                """Stage B matmuls + y^2 for tile j."""
                t0 = j * TN
                yB = [None, None]
                y2 = bwork.tile([128, 2, TN], BF16, tag="y2")
                for cb in range(2):
                    yB[cb] = ps_a.tile([128, 512], F32, tag="pa", name="yB")
                    for cp in range(4):
                        nc.tensor.matmul(
                            yB[cb][:, 0:TN],
                            w2T[:, cp, cb * 128:(cb + 1) * 128],
                            q_sm[:, cp, t0:t0 + TN],
                            start=(cp == 0), stop=False,
                            skip_group_check=True,
                        )
                    nc.tensor.matmul(
                        yB[cb][:, 0:TN],
                        broww[0:1, cb * 128:(cb + 1) * 128],
                        onesrow,
                        start=False, stop=True,
                        skip_group_check=True,
                    )
                    nc.scalar.activation(
                        out=y2[:, cb, :], in_=yB[cb][:, 0:TN], func=AF.Square
                    )
                smb = ps_sm.tile([128, 512], F32, tag="sm", name="smb")
                for nb in range(NB):
                    for cb in range(2):
                        nc.tensor.matmul(
                            smb[:, nb:nb + 1],
                            y2[:, cb, nb * 128:(nb + 1) * 128],
                            onecol_bf,
                            start=(cb == 0), stop=(cb == 1),
                            skip_group_check=True,
                        )
                return yB, smb

            def emit_b2(bl, j, yB, smb):
                """Stage B rms-out + store for tile j."""
                t0 = j * TN
                rrep_ps = rsqrt_compact(smb[:, 0:4], smb, bwork, "rinv")
                rinvrep = bwork.tile([128, TN], F32, tag="rinvsb")
                nc.scalar.copy(out=rinvrep, in_=rrep_ps[:, 0:TN])
                yout = bwork.tile([128, 2, TN], F32, tag="yout")
                for cb in range(2):
                    nc.vector.scalar_tensor_tensor(
                        out=yout[:, cb, :],
                        in0=yB[cb][:, 0:TN],
                        scalar=g2c[:, cb:cb + 1],
                        in1=rinvrep,
                        op0=ALU.mult, op1=ALU.mult,
                    )
                nc.sync.dma_start(out=ov[bl, :, :, t0:t0 + TN], in_=yout)

            # ---------------- main pipeline (software pipelined) ----------------
            xn_fulls = []
            ctxs = {}
            q_sms = []
            w2Ts = [None] * BL
            for bl in range(BL):
                xn_full = persist.tile([128, 2, N], BF16, tag="xnfull")
                xn_fulls.append(xn_full)
                q_sm = persist.tile([128, 4, N], BF16, tag="qsm")
                q_sms.append(q_sm)
                ctx = ps_ctx.tile([128, 512], F32, tag="ctx")
                nc.vector.memset(ctx, 0.0)
                ctxs[bl] = ctx

                xts = {}
                fr = {}
                bst = {}
                for j in range(NT + 3):
                    # overlap the previous batch's epilogue with our warmup
                    if bl > 0 and j == 2:
                        w2Ts[bl - 1] = emit_epilogue(bl - 1, ctxs[bl - 1])
                    if j < NT:
                        if bl == 0 and j in xts_pre:
                            xts[j] = xts_pre.pop(j)
                        else:
                            xts[j] = emit_load(bl, j)
                    if j >= 1 and j - 1 < NT:
                        fr[j - 1] = emit_front1(bl, j - 1, xts[j - 1])
                    if bl > 0 and j >= 3 and j - 3 in bst:
                        emit_b2(bl - 1, j - 3, *bst.pop(j - 3))
                    if j >= 2 and j - 2 < NT:
                        eq = emit_qproj(bl, j - 2, xn_full)
                        emit_qdiv(bl, j - 2, *eq, q_sm)
                    if j >= 1 and j - 1 < NT:
                        emit_front2(bl, j - 1, xts.pop(j - 1), fr.pop(j - 1), xn_full)
                    if j >= 3:
                        emit_kvpath(bl, j - 3, xn_full, ctx)
                    if bl > 0 and 2 <= j and j - 2 < NT:
                        bst[j - 2] = emit_b1(bl - 1, j - 2, q_sms[bl - 1],
                                             w2Ts[bl - 1])
                if bl > 0:
                    for jj in sorted(bst):
                        emit_b2(bl - 1, jj, *bst.pop(jj))
                if bl == BL - 1:
                    w2Ts[bl] = emit_epilogue(bl, ctx)
            bst = {}
            for j in range(NT + 2):
                if j >= 2:
                    emit_b2(BL - 1, j - 2, *bst.pop(j - 2))
                if j < NT:
                    bst[j] = emit_b1(BL - 1, j, q_sms[BL - 1], w2Ts[BL - 1])

    nc.finalize()
    return nc


_NC_CACHE = None


def kernel(x, g1, Wqkv, Wout, bout, g2):
    global _NC_CACHE
    x = np.ascontiguousarray(np.asarray(x, dtype=np.float32))
    g1 = np.asarray(g1, dtype=np.float32)
    Wqkv = np.ascontiguousarray(np.asarray(Wqkv, dtype=np.float32))
    Wout = np.ascontiguousarray(np.asarray(Wout, dtype=np.float32))
    bout = np.asarray(bout, dtype=np.float32)
    g2 = np.asarray(g2, dtype=np.float32)

    b, c, H, W = x.shape
    xr = x.reshape(b, c, H * W)
    if _NC_CACHE is None:
        _NC_CACHE = build_kernel()
    nc = _NC_CACHE

    in_maps = []
    for core in range(8):
        in_maps.append({
            "x": np.ascontiguousarray(xr[core * BL:(core + 1) * BL]),
            "Wqkv": Wqkv, "Wout": Wout, "bout": bout, "g1": g1, "g2": g2,
        })
    res = run_bass_kernel_spmd(nc, in_maps, core_ids=list(range(8)))
    out = np.concatenate([m["out"] for m in res.results], axis=0)
    return out.reshape(b, c, H, W).astype(np.float32)


if __name__ == "__main__":
    nc = build_kernel()
    from concourse.timeline_sim import TimelineSim
    print("sim ns:", TimelineSim(nc, trace=False).simulate())


# revision 6
# speedup vs baseline: 1.2671x; 1.1082x over previous
"""LinearAttention kernel for Trainium2, 8 NeuronCores, data-parallel over batch.

v2: fp8 DoubleRow matmuls; ACT runs only {Exp, Square, Copy} (single act table);
rsqrt/divide done on DVE via pow/divide ALU ops; per-token reductions computed
compactly ([128,4] per tile) and replicated through tiny PE outer-products.

Scaling bookkeeping (all cancel in the final rms-norm):
  Wqkv stored *16 (fp8 range), exp() applied with scale=1/16, bias=-1.5.
  q_sm stored *16 (block-diag mask folds 1/16 into the replicated qden).
  W2 stored = (Wout @ ctx^T / kden) * 0.5  (= 64 * W2_true; v carries its 16)
  => y psum = 1024 * y_true; bout row scaled *1024; rms-norm removes it all.

Per-batch layouts:
  x, xn     [c=2x128, n]    channels on partitions (xn fp8)
  q psum    [128, TN]       per ob; expq fp32r SBUF; q_sm fp8 = expq/qdrep
  kT/vT     [tok 128, 512]  per 128-token block (k and v each one DR matmul)
  ctx psum  [66, 8*64]      rows 0..63 = sum exp(k)*v, row 64 = kden
  y psum    [c 2x128, TN]   stage B in [c, n]; per-token rsqrt replicated via
                            transpose + outer-products; final scale on Pool.
"""

import numpy as np

import concourse.bass as bass
import concourse.tile as tile
from concourse import bacc, mybir
from concourse.bass_utils import run_bass_kernel_spmd
from concourse.masks import make_identity

F32 = mybir.dt.float32
F32R = mybir.dt.float32r
BF16 = mybir.dt.bfloat16
FP8 = mybir.dt.float8e4
U32 = mybir.dt.uint32

AF = mybir.ActivationFunctionType
ALU = mybir.AluOpType
DR = mybir.MatmulPerfMode.DoubleRow

B = 16          # total batches
BL = 2          # batches per core
C = 256         # in channels
HID = 512       # heads * dim_head
HEADS = 8
DH = 64         # dim head
N = 4096        # tokens (64*64)
TN = 512        # token tile
NT = N // TN    # 8 token tiles per batch
NB = TN // 128  # 4 128-token blocks per tile


def build_kernel():
    nc = bacc.Bacc("TRN2", target_bir_lowering=False, debug=False, num_devices=8)

    x_d = nc.dram_tensor("x", [BL, C, N], F32, kind="ExternalInput").ap()
    wqkv_d = nc.dram_tensor("Wqkv", [3 * HID, C], F32, kind="ExternalInput").ap()
    wout_d = nc.dram_tensor("Wout", [C, HID], F32, kind="ExternalInput").ap()
    bout_d = nc.dram_tensor("bout", [C], F32, kind="ExternalInput").ap()
    g1_d = nc.dram_tensor("g1", [C], F32, kind="ExternalInput").ap()
    g2_d = nc.dram_tensor("g2", [C], F32, kind="ExternalInput").ap()
    o_d = nc.dram_tensor("out", [BL, C, N], F32, kind="ExternalOutput").ap()

    xv = x_d.rearrange("b (cb p) n -> b p cb n", cb=2)
    ov = o_d.rearrange("b (cb p) n -> b p cb n", cb=2)

    with tile.TileContext(nc) as tc:
        with (
            tc.tile_pool(name="const", bufs=1) as const,
            tc.tile_pool(name="wt", bufs=1) as wt,
            tc.tile_pool(name="stage", bufs=1) as stage,
            tc.tile_pool(name="xin", bufs=3) as xin,
            tc.tile_pool(name="front", bufs=2) as front,
            tc.tile_pool(name="qwork", bufs=2) as qwork,
            tc.tile_pool(name="kvw", bufs=3) as kvw,
            tc.tile_pool(name="persist", bufs=2) as persist,
            tc.tile_pool(name="bwork", bufs=2) as bwork,
            tc.tile_pool(name="ps_a", bufs=4, space="PSUM") as ps_a,
            tc.tile_pool(name="ps_sm", bufs=1, space="PSUM") as ps_sm,
            tc.tile_pool(name="ps_kv", bufs=2, space="PSUM") as ps_kv,
            tc.tile_pool(name="ps_ctx", bufs=1, space="PSUM") as ps_ctx,
        ):
            # ---------------- constants ----------------
            ident = const.tile([128, 128], F32)
            make_identity(nc, ident)

            ones2_f8 = const.tile([128, 2, 1], FP8)
            nc.gpsimd.memset(ones2_f8, 1.0)
            onerow_bf = const.tile([1, 128], BF16)
            nc.gpsimd.memset(onerow_bf, 1.0)

            # block-diag [128,128] fp32r, value 1/16: replicated per-head sums
            bd_f = const.tile([128, 128], F32)
            nc.gpsimd.memset(bd_f, 0.0)
            nc.gpsimd.memset(bd_f[0:64, 0:64], 1.0 / 16.0)
            nc.gpsimd.memset(bd_f[64:128, 64:128], 1.0 / 16.0)
            bdr = const.tile([128, 128], BF16)
            nc.vector.tensor_copy(out=bdr, in_=bd_f)

            onecol_bf = const.tile([128, 1], BF16)
            nc.gpsimd.memset(onecol_bf, 1.0)
            onesrow = const.tile([1, 512], BF16)
            nc.gpsimd.memset(onesrow, 1.0)
            biasm = const.tile([128, 1], F32)
            nc.gpsimd.memset(biasm, -1.5)

            g1c = const.tile([128, 2], F32)
            nc.sync.dma_start(out=g1c, in_=g1_d.rearrange("(cb p) -> p cb", cb=2))
            g1s = const.tile([128, 2], F32)
            nc.vector.tensor_scalar_mul(out=g1s, in0=g1c, scalar1=16.0)
            g2c = const.tile([128, 2], F32)
            nc.sync.dma_start(out=g2c, in_=g2_d.rearrange("(cb p) -> p cb", cb=2))

            # bout as a row, scaled by 1024 (total y scale), bf16
            brow = const.tile([1, 256], F32)
            nc.sync.dma_start(out=brow, in_=bout_d.rearrange("(a c) -> a c", a=1))
            broww = const.tile([1, 256], BF16)
            nc.vector.tensor_scalar_mul(out=broww, in0=brow, scalar1=1024.0)

            # prefetch the first x tiles before the weight-prep flood
            xts_pre = {}
            for jpre in range(2):
                xt_pre = xin.tile([128, 2, TN], F32, tag="xt", name="xt_pre")
                nc.sync.dma_start(out=xt_pre, in_=xv[0, :, :, jpre * TN:(jpre + 1) * TN])
                xts_pre[jpre] = xt_pre

            # ---------------- weights ----------------
            # Wqkv [1536, 256] -> wqkvT fp8 [c(2x128), cb, 1536], rows *g1*16
            wq_nat = stage.tile([128, 12, 256], F32, tag="wnat")
            nc.sync.dma_start(
                out=wq_nat, in_=wqkv_d.rearrange("(ob p) c -> p ob c", p=128)
            )
            wqkvTb = wt.tile([128, 2, 1536], BF16)
            for ob in range(12):
                for cb in range(2):
                    pt = ps_a.tile([128, 512], F32, tag="pa", name="pt")
                    nc.tensor.transpose(
                        pt[:, 0:128], wq_nat[:, ob, cb * 128:(cb + 1) * 128], ident
                    )
                    nc.vector.tensor_scalar_mul(
                        out=wqkvTb[:, cb, ob * 128:(ob + 1) * 128],
                        in0=pt[:, 0:128],
                        scalar1=g1s[:, cb:cb + 1],
                    )
            # Wout [256, 512] -> woutT [e=64, h, 256] fp32r
            wo_nat = stage.tile([128, 2, 512], F32, tag="wnat")
            nc.sync.dma_start(
                out=wo_nat, in_=wout_d.rearrange("(ob p) h -> p ob h", p=128)
            )
            woutT = wt.tile([64, 8, 256], F32R)
            for h in range(HEADS):
                for ob in range(2):
                    pt = ps_a.tile([128, 512], F32, tag="pa", name="pt")
                    nc.tensor.transpose(
                        pt[0:64, 0:128], wo_nat[:, ob, h * 64:(h + 1) * 64], ident
                    )
                    nc.vector.tensor_copy(
                        out=woutT[:, h, ob * 128:(ob + 1) * 128], in_=pt[0:64, 0:128]
                    )
            scl_f = const.tile([1, 2], F32)
            nc.gpsimd.memset(scl_f, 0.5)
            scl_r = const.tile([1, 2], F32R)
            nc.vector.tensor_copy(out=scl_r, in_=scl_f)  # kden transpose helper

            # ---------------- helpers ----------------
            MAGIC = 0x5F3759DF + 0x02000000  # rsqrt seed for m = ssq/256
            MAGIC_SUB = 0x7FFFFFFF - MAGIC  # overflow-free: C-(i>>1) = ((i>>1)^0x7fffffff) - this

            def rsqrt_compact(cT, sm, pool, tag):
                """cT [128,4] f32 psum of per-token ssq -> replicated
                [128, 512] f32 psum with 16/sqrt(ssq) (bit-hack + 1 Newton).
                sm is the host psum tile for the transpose scratch."""
                t1 = pool.tile([128, 4], U32, tag=tag + "t1", name="t1")
                nc.vector.tensor_scalar(
                    out=t1, in0=cT.bitcast(U32), scalar1=1, scalar2=0x7FFFFFFF,
                    op0=ALU.logical_shift_right, op1=ALU.bitwise_xor,
                )
                y0 = pool.tile([128, 4], U32, tag=tag + "y0", name="y0")
                nc.vector.tensor_scalar(
                    out=y0, in0=t1, scalar1=MAGIC_SUB, scalar2=None, op0=ALU.subtract
                )
                y0f = y0.bitcast(F32)
                t2 = pool.tile([128, 4], F32, tag=tag + "t2", name="t2")
                nc.vector.tensor_mul(t2, y0f, y0f)
                nc.vector.tensor_mul(t2, t2, cT)
                t3 = pool.tile([128, 4], F32, tag=tag + "t3", name="t3")
                nc.vector.tensor_scalar(
                    out=t3, in0=t2, scalar1=-1.0 / 512.0, scalar2=1.5,
                    op0=ALU.mult, op1=ALU.add,
                )
                y1 = pool.tile([128, 4], F32, tag=tag + "y1", name="y1")
                nc.vector.tensor_mul(y1, y0f, t3)
                nc.vector.tensor_mul(t2, y1, y1)
                nc.vector.tensor_mul(t2, t2, cT)
                nc.vector.tensor_scalar(
                    out=t3, in0=t2, scalar1=-1.0 / 512.0, scalar2=1.5,
                    op0=ALU.mult, op1=ALU.add,
                )
                vT = pool.tile([128, 4], F32, tag=tag + "vT", name="vT")
                nc.vector.tensor_mul(vT, y1, t3)
                # replicate: 4 partition-0 transposes -> one sbuf row -> bcast
                for nb in range(NB):
                    nc.tensor.transpose(
                        sm[0:1, nb * 128:(nb + 1) * 128], vT[:, nb:nb + 1], ident
                    )
                vrow = pool.tile([1, 512], F32, tag=tag + "row", name="vrow", bufs=1)
                nc.scalar.copy(out=vrow, in_=sm[0:1, 0:512])
                vrep = pool.tile([128, 512], F32, tag=tag + "rep", name="vrep", bufs=1)
                nc.gpsimd.partition_broadcast(vrep, vrow)
                return vrep

            def emit_load(bl, j):
                """Prefetch x tile j."""
                xt = xin.tile([128, 2, TN], F32, tag="xt")
                nc.sync.dma_start(out=xt, in_=xv[bl, :, :, j * TN:(j + 1) * TN])
                return xt

            def emit_front1(bl, j, xt):
                """x^2 + compact per-token ssq for tile j."""
                x2 = front.tile([128, 2, TN], FP8, tag="x2")
                nc.vector.tensor_mul(x2, xt, xt)
                sm = ps_sm.tile([128, 512], F32, tag="sm", name="sm")
                for nb in range(NB):
                    nc.tensor.matmul(
                        sm[:, nb:nb + 1],
                        x2[:, :, nb * 128:(nb + 1) * 128],
                        ones2_f8,
                        start=True, stop=True,
                        perf_mode=DR,
                        skip_group_check=True,
                    )
                return sm

            def emit_front2(bl, j, xt, sm, xn_full):
                """rsqrt + normalize tile j into xn_full (fp8)."""
                t0 = j * TN
                sinvrep = rsqrt_compact(sm[:, 0:4], sm, front, "sinv")
                for cb in range(2):
                    nc.vector.tensor_mul(
                        xn_full[:, cb, t0:t0 + TN], xt[:, cb, :], sinvrep[:, 0:TN]
                    )

            def emit_qproj(bl, j, xn_full):
                """S1: q projection + exp for tile j -> (expq, pq psum tiles)."""
                t0 = j * TN
                expq = qwork.tile([128, 4, TN], BF16, tag="expq")
                pqs = []
                for ob in range(4):
                    pq = ps_a.tile([128, 512], F32, tag="pa", name="pq")
                    pqs.append(pq)
                    for cb in range(2):
                        nc.tensor.matmul(
                            pq[:, 0:TN],
                            wqkvTb[:, cb, ob * 128:(ob + 1) * 128],
                            xn_full[:, cb, t0:t0 + TN],
                            start=(cb == 0), stop=(cb == 1),
                        )
                    nc.scalar.activation(
                        out=expq[:, ob, :], in_=pq[:, 0:TN],
                        func=AF.Exp, scale=1.0 / 16.0, bias=biasm,
                    )
                return expq, pqs

            def emit_qdiv(bl, j, expq, pqs, q_sm):
                """S2: replicated per-head denominators (into the freed pq
                banks), reciprocal to SBUF, multiply on Pool."""
                t0 = j * TN
                for ob in range(4):
                    nc.tensor.matmul(
                        pqs[ob][:, 0:TN], bdr, expq[:, ob, :],
                        start=True, stop=True,
                    )
                    qdinv = qwork.tile([128, TN], F32R, tag="qdinv")
                    with nc.allow_low_precision(reason="q softmax denom"):
                        nc.vector.reciprocal(out=qdinv, in_=pqs[ob][:, 0:TN])
                    nc.gpsimd.tensor_mul(
                        q_sm[:, ob, t0:t0 + TN], expq[:, ob, :], qdinv
                    )

            def emit_kvpath(bl, j, xn_full, ctx):
                """kv projection + exp(k) + ctx accumulation for tile j."""
                t0 = j * TN
                for half in range(2):  # two 2-block pairs
                    expk2 = kvw.tile([128, 2, 8, 64], BF16, tag="expk")
                    vt2 = kvw.tile([128, 2, 8, 66], BF16, tag="vt")
                    nc.gpsimd.memset(vt2[:, :, :, 64:65], 1.0)
                    nc.gpsimd.memset(vt2[:, :, :, 65:66], 0.0)
                    for bi in range(2):
                        nb = half * 2 + bi
                        pk = ps_kv.tile([128, 512], F32, tag="pkv", name="pk")
                        for cb in range(2):
                            nc.tensor.matmul(
                                pk,
                                xn_full[:, cb, t0 + nb * 128:t0 + (nb + 1) * 128],
                                wqkvTb[:, cb, 512:1024],
                                start=(cb == 0), stop=(cb == 1),
                            )
                        pv = ps_kv.tile([128, 512], F32, tag="pkv", name="pv")
                        for cb in range(2):
                            nc.tensor.matmul(
                                pv,
                                xn_full[:, cb, t0 + nb * 128:t0 + (nb + 1) * 128],
                                wqkvTb[:, cb, 1024:1536],
                                start=(cb == 0), stop=(cb == 1),
                            )
                        nc.scalar.activation(
                            out=expk2[:, bi], in_=pk,
                            func=AF.Exp, scale=1.0 / 16.0, bias=biasm,
                        )
                        nc.scalar.copy(
                            out=vt2[:, bi, :, 0:64],
                            in_=pv.rearrange("p (h e) -> p h e", h=8),
                        )
                    gpair = j * 2 + half
                    for bi in range(2):
                        for h in range(HEADS):
                            nc.tensor.matmul(
                                ctx[0:66, h * 64:(h + 1) * 64],
                                vt2[:, bi, h, :],
                                expk2[:, bi, h, :],
                                start=False,
                                stop=(gpair == 2 * NT - 1 and bi == 1),
                                skip_group_check=True,
                            )

            def emit_epilogue(bl, ctx):
                """Build w2T fp8 [128, 4, 256] = (Wout_h @ ctx_h^T / kden * .5)^T"""
                ctx_sb = persist.tile([64, 512], F32R, tag="ctxsb")
                nc.vector.tensor_copy(out=ctx_sb, in_=ctx[0:64, :])
                kdrow = persist.tile([1, 512], F32R, tag="kdrow")
                with nc.allow_low_precision(reason="k softmax denominators"):
                    nc.vector.reciprocal(out=kdrow, in_=ctx[64:65, :])
                # transpose kden row -> columns [64, 8]
                pkd = ps_a.tile([128, 512], F32, tag="pa", name="pkd")
                for h in range(HEADS):
                    nc.tensor.matmul(
                        pkd[0:64, 2 * h:2 * h + 2],
                        kdrow[0:1, h * 64:(h + 1) * 64],
                        scl_r,
                        start=True, stop=True,
                        skip_group_check=True,
                    )
                kdcol = persist.tile([64, 8, 1], F32, tag="kdcol")
                pkd_v = pkd[0:64, 0:16].rearrange("p (h t) -> p h t", t=2)
                nc.vector.tensor_copy(out=kdcol, in_=pkd_v[:, :, 0:1])
                w2stg = persist.tile([64, 8, 256], BF16, tag="w2stg")
                for h in range(HEADS):
                    pw2 = ps_a.tile([128, 512], F32, tag="pa", name="pw2")
                    nc.tensor.matmul(
                        pw2[0:64, 0:256],
                        ctx_sb[:, h * 64:(h + 1) * 64],
                        woutT[:, h, :],
                        start=True, stop=True,
                    )
                    nc.vector.tensor_scalar_mul(
                        out=w2stg[:, h, :],
                        in0=pw2[0:64, 0:256],
                        scalar1=kdcol[:, h, :],
                    )
                w2T = persist.tile([128, 4, 256], BF16, tag="w2T")
                for h in range(HEADS):
                    nc.sync.dma_start(
                        out=w2T[(h % 2) * 64:(h % 2) * 64 + 64, h // 2, :],
                        in_=w2stg[:, h, :],
                    )
                return w2T

            def emit_b1(bl, j, q_sm, w2T):
                """Stage B matmuls + y^2 for tile j."""
                t0 = j * TN
                yB = [None, None]
                y2 = bwork.tile([128, 2, TN], BF16, tag="y2")
                for cb in range(2):
                    yB[cb] = ps_a.tile([128, 512], F32, tag="pa", name="yB")
                    for cp in range(4):
                        nc.tensor.matmul(
                            yB[cb][:, 0:TN],
                            w2T[:, cp, cb * 128:(cb + 1) * 128],
                            q_sm[:, cp, t0:t0 + TN],
                            start=(cp == 0), stop=False,
                            skip_group_check=True,
                        )
                    nc.tensor.matmul(
                        yB[cb][:, 0:TN],
                        broww[0:1, cb * 128:(cb + 1) * 128],
                        onesrow,
                        start=False, stop=True,
                        skip_group_check=True,
                    )
                    nc.scalar.activation(
                        out=y2[:, cb, :], in_=yB[cb][:, 0:TN], func=AF.Square
                    )
                smb = ps_sm.tile([128, 512], F32, tag="sm", name="smb")
                for nb in range(NB):
                    for cb in range(2):
                        nc.tensor.matmul(
                            smb[:, nb:nb + 1],
                            y2[:, cb, nb * 128:(nb + 1) * 128],
                            onecol_bf,
                            start=(cb == 0), stop=(cb == 1),
                            skip_group_check=True,
                        )
                return yB, smb

            def emit_b2(bl, j, yB, smb):
                """Stage B rms-out + store for tile j."""
                t0 = j * TN
                rinvrep = rsqrt_compact(smb[:, 0:4], smb, bwork, "rinv")
                yout = bwork.tile([128, 2, TN], F32, tag="yout")
                for cb in range(2):
                    nc.vector.scalar_tensor_tensor(
                        out=yout[:, cb, :],
                        in0=yB[cb][:, 0:TN],
                        scalar=g2c[:, cb:cb + 1],
                        in1=rinvrep,
                        op0=ALU.mult, op1=ALU.mult,
                    )
                nc.sync.dma_start(out=ov[bl, :, :, t0:t0 + TN], in_=yout)

            # ---------------- main pipeline (software pipelined) ----------------
            xn_fulls = []
            ctxs = {}
            q_sms = []
            w2Ts = [None] * BL
            for bl in range(BL):
                xn_full = persist.tile([128, 2, N], BF16, tag="xnfull")
                xn_fulls.append(xn_full)
                q_sm = persist.tile([128, 4, N], BF16, tag="qsm")
                q_sms.append(q_sm)
                ctx = ps_ctx.tile([128, 512], F32, tag="ctx")
                nc.vector.memset(ctx, 0.0)
                ctxs[bl] = ctx

                xts = {}
                fr = {}
                bst = {}
                for j in range(NT + 3):
                    # overlap the previous batch's epilogue with our warmup
                    if bl > 0 and j == 2:
                        w2Ts[bl - 1] = emit_epilogue(bl - 1, ctxs[bl - 1])
                    if j < NT:
                        if bl == 0 and j in xts_pre:
                            xts[j] = xts_pre.pop(j)
                        else:
                            xts[j] = emit_load(bl, j)
                    if j >= 1 and j - 1 < NT:
                        fr[j - 1] = emit_front1(bl, j - 1, xts[j - 1])
                    if bl > 0 and j >= 3 and j - 3 in bst:
                        emit_b2(bl - 1, j - 3, *bst.pop(j - 3))
                    if j >= 2 and j - 2 < NT:
                        eq = emit_qproj(bl, j - 2, xn_full)
                        emit_qdiv(bl, j - 2, *eq, q_sm)
                    if j >= 1 and j - 1 < NT:
                        emit_front2(bl, j - 1, xts.pop(j - 1), fr.pop(j - 1), xn_full)
                    if j >= 3:
                        emit_kvpath(bl, j - 3, xn_full, ctx)
                    if bl > 0 and 2 <= j and j - 2 < NT:
                        bst[j - 2] = emit_b1(bl - 1, j - 2, q_sms[bl - 1],
                                             w2Ts[bl - 1])
                if bl > 0:
                    for jj in sorted(bst):
                        emit_b2(bl - 1, jj, *bst.pop(jj))
                if bl == BL - 1:
                    w2Ts[bl] = emit_epilogue(bl, ctx)
            bst = {}
            for j in range(NT + 2):
                if j >= 2:
                    emit_b2(BL - 1, j - 2, *bst.pop(j - 2))
                if j < NT:
                    bst[j] = emit_b1(BL - 1, j, q_sms[BL - 1], w2Ts[BL - 1])

    nc.finalize()
    return nc


_NC_CACHE = None


def kernel(x, g1, Wqkv, Wout, bout, g2):
    global _NC_CACHE
    x = np.ascontiguousarray(np.asarray(x, dtype=np.float32))
    g1 = np.asarray(g1, dtype=np.float32)
    Wqkv = np.ascontiguousarray(np.asarray(Wqkv, dtype=np.float32))
    Wout = np.ascontiguousarray(np.asarray(Wout, dtype=np.float32))
    bout = np.asarray(bout, dtype=np.float32)
    g2 = np.asarray(g2, dtype=np.float32)

    b, c, H, W = x.shape
    xr = x.reshape(b, c, H * W)
    if _NC_CACHE is None:
        _NC_CACHE = build_kernel()
    nc = _NC_CACHE

    in_maps = []
    for core in range(8):
        in_maps.append({
            "x": np.ascontiguousarray(xr[core * BL:(core + 1) * BL]),
            "Wqkv": Wqkv, "Wout": Wout, "bout": bout, "g1": g1, "g2": g2,
        })
    res = run_bass_kernel_spmd(nc, in_maps, core_ids=list(range(8)))
    out = np.concatenate([m["out"] for m in res.results], axis=0)
    return out.reshape(b, c, H, W).astype(np.float32)


if __name__ == "__main__":
    nc = build_kernel()
    from concourse.timeline_sim import TimelineSim
    print("sim ns:", TimelineSim(nc, trace=False).simulate())
